# revision 11
# baseline (speedup 1.0000x reference)
"""Trainium2 Bass kernel for nn_ByteGridModel (dense_cnn).

Sharding: pure data-parallel over batch B=8 -> 8 cores, one batch item per
core, no collectives. Weights replicated (streamed per layer, double
buffered).

Per-core layout: channels on partitions, h = [128, NT=4, S=256] fp32r
resident in SBUF (one tile, four 128-channel groups).

v2: mixer broadcast-products reordered so every operand's innermost free
    dim is stride-1 bf16 -> DVE 2x_1P mode; ACT sqrt + DVE fast-reciprocal
    for rmsnorm; transposed-v trick for the global mixer.
v3: PE became the bottleneck (serial LDWEIGHTS per matmul + power-governor
    duty cycle capping PE at ~1.37GHz avg). Halve matmul count: process
    h-tile PAIRS with N=512 accumulation psums; residual adds on DVE
    (reading the [128,512] psum once); merge per-tile DVE muls and ACT
    squares into single wide ops.
"""

import numpy as np
import ml_dtypes

import concourse.bacc as bacc
import concourse.bass as bass
import concourse.tile as tile
import concourse.mybir as mybir
from concourse.bass_utils import run_bass_kernel_spmd

B, S, H, GLU, VOC, L, CIN, BLK = 8, 256, 512, 1024, 256, 24, 320, 16
EPS = 1e-5
NT = H // 128  # 4 channel tiles
GT = GLU // 128  # 8 glu tiles
DV = 14  # of the 16 outer slices of each product tile, DVE does DV, GPSIMD 16-DV

F32 = mybir.dt.float32
F32R = mybir.dt.float32r
BF16 = mybir.dt.bfloat16
MULT = mybir.AluOpType.mult
ADD = mybir.AluOpType.add
AF = mybir.ActivationFunctionType

_PROG_CACHE = {}


def _bview(base, doff, free_dims):
    """View of a 2D sbuf AP with custom (possibly broadcast) free dims."""
    return bass.AP(
        tensor=base.tensor,
        offset=base.offset + doff,
        ap=[list(base.ap[0])] + [list(d) for d in free_dims],
    )


def build_program(n_layers=L, sim_compat=False):
    nc = bacc.Bacc("TRN2")

    x_d = nc.dram_tensor("x", [384, S], F32R, kind="ExternalInput")
    stw_d = nc.dram_tensor("stem_wT", [384, H], F32R, kind="ExternalInput")
    wv_d = nc.dram_tensor("wvT", [n_layers, H, GLU], BF16, kind="ExternalInput")
    wg_d = nc.dram_tensor("wgT", [n_layers, H, GLU], BF16, kind="ExternalInput")
    wo_d = nc.dram_tensor("woT", [n_layers, GLU, H], BF16, kind="ExternalInput")
    wl_d = nc.dram_tensor("wl", [n_layers, H, 256], BF16, kind="ExternalInput")
    wm_d = nc.dram_tensor("wm", [n_layers, H, 256], BF16, kind="ExternalInput")
    hw_d = nc.dram_tensor("headT", [H, VOC], BF16, kind="ExternalInput")
    id_d = nc.dram_tensor("ident", [128, 128], BF16, kind="ExternalInput")
    ones_d = nc.dram_tensor("ones_k", [128, 1], F32R, kind="ExternalInput")
    onesr_d = nc.dram_tensor("ones_m", [1, 128], F32R, kind="ExternalInput")
    out_d = nc.dram_tensor("out", [VOC, S], F32, kind="ExternalOutput")

    from contextlib import ExitStack

    with tile.TileContext(nc) as tc, ExitStack() as ctx:
        singles = ctx.enter_context(tc.tile_pool(name="singles", bufs=1))
        wpool = ctx.enter_context(tc.tile_pool(name="wpool", bufs=2))
        hpool = ctx.enter_context(tc.tile_pool(name="hpool", bufs=1))
        npool = ctx.enter_context(tc.tile_pool(name="npool", bufs=2))
        apool = ctx.enter_context(tc.tile_pool(name="apool", bufs=2))
        ppool = ctx.enter_context(tc.tile_pool(name="ppool", bufs=2))
        gpool = ctx.enter_context(tc.tile_pool(name="gpool", bufs=2))
        ps_n = ctx.enter_context(tc.tile_pool(name="ps_n", bufs=1, space="PSUM"))
        ps_m = ctx.enter_context(tc.tile_pool(name="ps_m", bufs=2, space="PSUM"))
        ps_g = ctx.enter_context(tc.tile_pool(name="ps_g", bufs=2, space="PSUM"))
        ps_o = ctx.enter_context(tc.tile_pool(name="ps_o", bufs=2, space="PSUM"))

        # ---- constants / stem operands ----
        ident = singles.tile([128, 128], BF16, tag="ident")
        nc.sync.dma_start(out=ident, in_=id_d[:])
        ones_k_st = singles.tile([128, 1], F32R, tag="ones_k_st")
        nc.sync.dma_start(out=ones_k_st, in_=ones_d[:])
        ones_k = singles.tile([128, 1], F32R, tag="ones_k")
        ones_m_st = singles.tile([1, 128], F32R, tag="ones_m_st")
        nc.sync.dma_start(out=ones_m_st, in_=onesr_d[:])
        ones_m = singles.tile([1, 128], F32R, tag="ones_m")
        eps_sb = singles.tile([1, 1], F32, tag="eps")
        nc.vector.memset(eps_sb, float(EPS))

        x_st = singles.tile([128, 3, S], F32R, tag="x_st")
        nc.sync.dma_start(out=x_st, in_=x_d[:].rearrange("(t p) s -> p t s", p=128))
        x_sb = singles.tile([128, 3, S], F32R, tag="x")
        stw_st = singles.tile([128, 3, H], F32R, tag="stw_st")
        nc.sync.dma_start(out=stw_st, in_=stw_d[:].rearrange("(t p) s -> p t s", p=128))
        stw_sb = singles.tile([128, 3, H], F32R, tag="stw")

        # Route fp32r matmul operands through a DVE copy so each matmul's
        # operand has an engine writer (a matmul can carry only one
        # cross-engine wait through walrus codegen). Touch bf16 weight DMAs
        # with ldweights for the same reason.
        with nc.allow_low_precision(reason="fp32r staging copies"):
            nc.vector.tensor_copy(out=ones_k, in_=ones_k_st)
            nc.vector.tensor_copy(out=ones_m, in_=ones_m_st)
            nc.vector.tensor_copy(out=x_sb, in_=x_st)
            nc.vector.tensor_copy(out=stw_sb, in_=stw_st)
        nc.tensor.ldweights(ident[:, 0:128])

        # ---- h (resident, fp32r, one tile of 4 channel groups) ----
        h_all = hpool.tile([128, NT, S], F32R, tag="h", name="h")

        # ---- stem: h = stem_w @ x ----
        for tp in (0, 2):
            pst = ps_o.tile([128, 2, S], F32, tag="po")
            for t2 in (0, 1):
                for kt in range(3):
                    nc.tensor.matmul(
                        pst[:, t2, :],
                        stw_sb[:, kt, (tp + t2) * 128 : (tp + t2 + 1) * 128],
                        x_sb[:, kt, :],
                        start=(kt == 0),
                        stop=(kt == 2),
                    )
            with nc.allow_low_precision(reason="h fp32r copyback"):
                nc.scalar.copy(out=h_all[:, tp : tp + 2, :], in_=pst)

        def rms_rb():
            """Returns PSUM [128, S] fp32 broadcast of 1/sqrt(mean(h^2)+eps)."""
            sq = apool.tile([128, NT, S], F32R, tag="sq")
            nc.scalar.square(sq, h_all)
            ms = ps_n.tile([1, S], F32, tag="ms")
            for t in range(NT):
                nc.tensor.matmul(
                    ms,
                    ones_k[:, 0:1],
                    sq[:, t, :],
                    start=(t == 0),
                    stop=(t == NT - 1),
                )
            stdv = npool.tile([1, S], F32, tag="stdv")
            nc.scalar.activation(
                stdv, ms, AF.Sqrt, bias=eps_sb[0:1, 0:1], scale=1.0 / H
            )
            rstd = npool.tile([1, S], F32, tag="rstd")
            nc.vector.reciprocal_approx_fast(out=rstd, in_=stdv)
            rstd_r = npool.tile([1, S], F32R, tag="rstd_r")
            with nc.allow_low_precision(reason="fp32r rstd for broadcast matmul"):
                nc.vector.tensor_copy(out=rstd_r, in_=rstd)
            rb = ps_n.tile([128, S], F32, tag="rb")
            nc.tensor.matmul(
                rb,
                ones_m[0:1, :],
                rstd_r[:],
                start=True,
                stop=True,
            )
            return rb

        def pair_acc_and_residual(prod, tp):
            """acc[128,512] = sum over the innermost-16 of the pair's prod via
            16 identity matmuls (N=512); then h[pair] += acc on DVE."""
            acc = ps_m.tile([128, 2, S], F32, tag="macc")
            for k in range(16):
                rhs = _bview(prod[:], k, [[4096, 2], [256, 16], [16, 16]])
                nc.tensor.matmul(
                    acc[:],
                    ident[:],
                    rhs,
                    start=(k == 0),
                    stop=(k == 15),
                )
            hp = h_all[:, tp : tp + 2, :]
            nc.vector.tensor_tensor(out=hp, in0=hp, in1=acc[:], op=ADD)

        for l in range(n_layers):
            wv_sb = wpool.tile([128, NT, GLU], BF16, tag="wv")
            nc.sync.dma_start(
                out=wv_sb, in_=wv_d[l].rearrange("(t p) o -> p t o", p=128)
            )
            wg_sb = wpool.tile([128, NT, GLU], BF16, tag="wg")
            nc.sync.dma_start(
                out=wg_sb, in_=wg_d[l].rearrange("(t p) o -> p t o", p=128)
            )
            wo_sb = wpool.tile([128, GT, H], BF16, tag="wo")
            nc.sync.dma_start(
                out=wo_sb, in_=wo_d[l].rearrange("(t p) c -> p t c", p=128)
            )
            wl_sb = wpool.tile([128, NT, 256], BF16, tag="wl")
            nc.sync.dma_start(
                out=wl_sb, in_=wl_d[l].rearrange("(t p) q -> p t q", p=128)
            )
            wm_sb = wpool.tile([128, NT, 256], BF16, tag="wm")
            nc.sync.dma_start(
                out=wm_sb, in_=wm_d[l].rearrange("(t p) q -> p t q", p=128)
            )
            nc.tensor.ldweights(wv_sb[:, 0, 0:128])
            nc.tensor.ldweights(wg_sb[:, 0, 0:128])
            nc.tensor.ldweights(wo_sb[:, 0, 0:128])

            # ---------- local mixer: out[c,16i+p] = sum_j Wl[c,p,j] u[c,16i+j]
            # prod free order (i, p, j): innermost j stride-1 for u, wl, out.
            rb = rms_rb()
            u_all = apool.tile([128, NT, S], BF16, tag="uall")
            nc.vector.tensor_tensor(
                out=u_all,
                in0=h_all,
                in1=_bview(rb[:], 0, [[0, NT], [1, S]]),
                op=MULT,
            )
            for tp in (0, 2):
                prod = ppool.tile([128, 2, 16, 16, 16], BF16, tag="prod")
                for t2 in (0, 1):
                    u_t = u_all[:, tp + t2, :]
                    wl_t = wl_sb[:, tp + t2, :]
                    uv_d = _bview(u_t, 0, [[16, DV], [0, 16], [1, 16]])
                    wl_v = _bview(wl_t, 0, [[0, DV], [16, 16], [1, 16]])
                    nc.vector.tensor_tensor(
                        out=prod[:, t2, 0:DV], in0=uv_d, in1=wl_v, op=MULT
                    )
                    uv_g = _bview(u_t, 16 * DV, [[16, 16 - DV], [0, 16], [1, 16]])
                    wl_g = _bview(wl_t, 0, [[0, 16 - DV], [16, 16], [1, 16]])
                    nc.gpsimd.tensor_tensor(
                        out=prod[:, t2, DV:16], in0=uv_g, in1=wl_g, op=MULT
                    )
                pair_acc_and_residual(prod, tp)

            # ---------- global mixer: out[c,16p+j] = sum_i Wg[c,p,i] v[c,16i+j]
            # vT[c,16j+i] written via transposed output AP; prod free (p, j, i).
            rb = rms_rb()
            vT_all = apool.tile([128, NT, S], BF16, tag="uall")
            nc.vector.tensor_tensor(
                out=_bview(vT_all[:], 0, [[256, NT], [16, 16], [1, 16]]),
                in0=_bview(h_all[:], 0, [[256, NT], [1, 16], [16, 16]]),
                in1=_bview(rb[:], 0, [[0, NT], [1, 16], [16, 16]]),
                op=MULT,
            )
            for tp in (0, 2):
                prod = ppool.tile([128, 2, 16, 16, 16], BF16, tag="prod")
                for t2 in (0, 1):
                    vt_t = vT_all[:, tp + t2, :]
                    wm_t = wm_sb[:, tp + t2, :]
                    vt_d = _bview(vt_t, 0, [[0, DV], [16, 16], [1, 16]])
                    wm_v = _bview(wm_t, 0, [[16, DV], [0, 16], [1, 16]])
                    nc.vector.tensor_tensor(
                        out=prod[:, t2, 0:DV], in0=vt_d, in1=wm_v, op=MULT
                    )
                    vt_g = _bview(vt_t, 0, [[0, 16 - DV], [16, 16], [1, 16]])
                    wm_g = _bview(
                        wm_t, 16 * DV, [[16, 16 - DV], [0, 16], [1, 16]]
                    )
                    nc.gpsimd.tensor_tensor(
                        out=prod[:, t2, DV:16], in0=vt_g, in1=wm_g, op=MULT
                    )
                pair_acc_and_residual(prod, tp)

            # ---------- GLU MLP
            rb = rms_rb()
            wn_all = apool.tile([128, NT, S], BF16, tag="wnall")
            nc.vector.tensor_tensor(
                out=wn_all,
                in0=h_all,
                in1=_bview(rb[:], 0, [[0, NT], [1, S]]),
                op=MULT,
            )
            gts = []
            for op_ in range(GT // 2):
                p1 = ps_g.tile([128, 2, S], F32, tag="pg")
                for oo in (0, 1):
                    for kt in range(NT):
                        nc.tensor.matmul(
                            p1[:, oo, :],
                            wv_sb[:, kt, (2 * op_ + oo) * 128 : (2 * op_ + oo + 1) * 128],
                            wn_all[:, kt, :],
                            start=(kt == 0),
                            stop=(kt == NT - 1),
                        )
                s1 = apool.tile([128, 2, S], BF16, tag="s1")
                if sim_compat:
                    sg = apool.tile([128, 2, S], BF16, tag="sg")
                    nc.scalar.activation(sg, p1, AF.Sigmoid)
                    nc.vector.tensor_tensor(out=s1, in0=sg, in1=p1, op=MULT)
                else:
                    nc.scalar.activation(s1, p1, AF.Silu)
                p3 = ps_g.tile([128, 2, S], F32, tag="pg")
                for oo in (0, 1):
                    for kt in range(NT):
                        nc.tensor.matmul(
                            p3[:, oo, :],
                            wg_sb[:, kt, (2 * op_ + oo) * 128 : (2 * op_ + oo + 1) * 128],
                            wn_all[:, kt, :],
                            start=(kt == 0),
                            stop=(kt == NT - 1),
                        )
                gt_ = gpool.tile([128, 2, S], BF16, tag=f"g{op_}")
                nc.vector.tensor_tensor(out=gt_, in0=s1, in1=p3, op=MULT)
                gts.append(gt_)
            for tp in (0, 2):
                po = ps_o.tile([128, 2, S], F32, tag="po")
                for t2 in (0, 1):
                    for ot in range(GT):
                        nc.tensor.matmul(
                            po[:, t2, :],
                            wo_sb[:, ot, (tp + t2) * 128 : (tp + t2 + 1) * 128],
                            gts[ot // 2][:, ot % 2, :],
                            start=(ot == 0),
                            stop=(ot == GT - 1),
                        )
                hp = h_all[:, tp : tp + 2, :]
                nc.vector.tensor_tensor(out=hp, in0=hp, in1=po[:], op=ADD)

        # ---------- head ----------
        hw_sb = singles.tile([128, NT, VOC], BF16, tag="hw")
        nc.sync.dma_start(out=hw_sb, in_=hw_d.rearrange("(t p) v -> p t v", p=128))
        nc.tensor.ldweights(hw_sb[:, 0, 0:128])
        rb = rms_rb()
        nrm_all = apool.tile([128, NT, S], BF16, tag="wnall")
        nc.vector.tensor_tensor(
            out=nrm_all,
            in0=h_all,
            in1=_bview(rb[:], 0, [[0, NT], [1, S]]),
            op=MULT,
        )
        po = ps_o.tile([128, 2, S], F32, tag="po")
        for mc in range(VOC // 128):
            for kt in range(NT):
                nc.tensor.matmul(
                    po[:, mc, :],
                    hw_sb[:, kt, mc * 128 : (mc + 1) * 128],
                    nrm_all[:, kt, :],
                    start=(kt == 0),
                    stop=(kt == NT - 1),
                )
        ot_sb = apool.tile([128, 2, S], F32, tag="osb")
        nc.scalar.copy(out=ot_sb, in_=po)
        for mc in range(VOC // 128):
            nc.sync.dma_start(
                out=out_d[mc * 128 : (mc + 1) * 128, :], in_=ot_sb[:, mc, :]
            )

    nc.compile()
    return nc


def _prep_inputs(inputs, n_layers=L):
    """Host-side weight folding + layout prep. Returns dict of np arrays."""
    f = lambda k: np.asarray(inputs[k], dtype=np.float32)
    x = f("x")
    stem_w = f("stem_w")  # [H, CIN]
    rl, rg, rf = f("rms_local"), f("rms_global"), f("rms_ffn")
    al, ag, am = f("alpha_local"), f("alpha_global"), f("alpha_mlp")
    w_local, w_global = f("w_local"), f("w_global")  # [L, H, BLK, BLK]
    wv, wg, wo = f("wv"), f("wg"), f("wo")
    head_rms, head_w = f("head_rms"), f("head_w")
    hls = np.float32(np.asarray(inputs["head_logit_scale"]))

    bf = ml_dtypes.bfloat16
    nl = n_layers

    # local: fold alpha_local * rms_local[c] into Wl[c,p,j]; layout [c, 16p+j]
    wl_h = (w_local[:nl] * al[:nl, None, None, None] * rl[:nl, :, None, None]).reshape(
        nl, H, 256
    )
    # global: Wg[c,p,i]; layout [c, 16p+i]
    wm_h = (w_global[:nl] * ag[:nl, None, None, None] * rg[:nl, :, None, None]).reshape(
        nl, H, 256
    )
    # GLU: fold rms_ffn into wv/wg columns; alpha_mlp into wo
    wvT = np.ascontiguousarray(
        np.transpose(wv[:nl] * rf[:nl, None, :], (0, 2, 1))
    )  # [L, H, GLU]
    wgT = np.ascontiguousarray(np.transpose(wg[:nl] * rf[:nl, None, :], (0, 2, 1)))
    woT = np.ascontiguousarray(
        np.transpose(wo[:nl] * am[:nl, None, None], (0, 2, 1))
    )  # [L, GLU, H]
    headT = np.ascontiguousarray((head_w * head_rms[None, :] * hls).T)  # [H, VOC]

    stw_pad = np.zeros((384, H), np.float32)
    stw_pad[:CIN] = stem_w.T
    common = {
        "stem_wT": stw_pad,  # [384, H] zero-padded
        "wvT": wvT.astype(bf),
        "wgT": wgT.astype(bf),
        "woT": woT.astype(bf),
        "wl": wl_h.astype(bf),
        "wm": wm_h.astype(bf),
        "headT": headT.astype(bf),
        "ident": np.eye(128, dtype=bf),
        "ones_k": np.ones((128, 1), np.float32),
        "ones_m": np.ones((1, 128), np.float32),
    }
    per_core = []
    for b in range(B):
        xp = np.zeros((384, S), np.float32)
        xp[:CIN] = x[b, :, 0, :]
        per_core.append(dict(common, x=xp))
    return per_core


def run(inputs, n_layers=L, trace=False):
    key = n_layers
    if key not in _PROG_CACHE:
        _PROG_CACHE[key] = build_program(n_layers)
    nc = _PROG_CACHE[key]
    in_maps = _prep_inputs(inputs, n_layers)
    res = run_bass_kernel_spmd(nc, in_maps, core_ids=list(range(B)), trace=trace)
    out = np.stack([r["out"] for r in res.results])  # [B, VOC, S]
    return out[:, :, None, :].astype(np.float32), res


def kernel(**inputs):
    out, _ = run(inputs, L, trace=False)
    return out


# revision 14
# speedup vs baseline: 1.0012x; 1.0012x over previous
"""Trainium2 Bass kernel for nn_ByteGridModel (dense_cnn).

Sharding: pure data-parallel over batch B=8 -> 8 cores, one batch item per
core, no collectives. Weights replicated (streamed per layer, double
buffered).

Per-core layout: channels on partitions, h = [128, NT=4, S=256] fp32r
resident in SBUF (one tile, four 128-channel groups).

v2: mixer broadcast-products reordered so every operand's innermost free
    dim is stride-1 bf16 -> DVE 2x_1P mode; ACT sqrt + DVE fast-reciprocal
    for rmsnorm; transposed-v trick for the global mixer.
v3: PE became the bottleneck (serial LDWEIGHTS per matmul + power-governor
    duty cycle capping PE at ~1.37GHz avg). Halve matmul count: process
    h-tile PAIRS with N=512 accumulation psums; residual adds on DVE
    (reading the [128,512] psum once); merge per-tile DVE muls and ACT
    squares into single wide ops.
"""

import numpy as np
import ml_dtypes

import concourse.bacc as bacc
import concourse.bass as bass
import concourse.tile as tile
import concourse.mybir as mybir
from concourse.bass_utils import run_bass_kernel_spmd

B, S, H, GLU, VOC, L, CIN, BLK = 8, 256, 512, 1024, 256, 24, 320, 16
EPS = 1e-5
NT = H // 128  # 4 channel tiles
GT = GLU // 128  # 8 glu tiles
DV = 14  # of the 16 outer slices of each product tile, DVE does DV, GPSIMD 16-DV

F32 = mybir.dt.float32
F32R = mybir.dt.float32r
BF16 = mybir.dt.bfloat16
MULT = mybir.AluOpType.mult
ADD = mybir.AluOpType.add
AF = mybir.ActivationFunctionType

_PROG_CACHE = {}


def _bview(base, doff, free_dims):
    """View of a 2D sbuf AP with custom (possibly broadcast) free dims."""
    return bass.AP(
        tensor=base.tensor,
        offset=base.offset + doff,
        ap=[list(base.ap[0])] + [list(d) for d in free_dims],
    )


def build_program(n_layers=L, sim_compat=False):
    nc = bacc.Bacc("TRN2")

    x_d = nc.dram_tensor("x", [384, S], F32R, kind="ExternalInput")
    stw_d = nc.dram_tensor("stem_wT", [384, H], F32R, kind="ExternalInput")
    wv_d = nc.dram_tensor("wvT", [n_layers, H, GLU], BF16, kind="ExternalInput")
    wg_d = nc.dram_tensor("wgT", [n_layers, H, GLU], BF16, kind="ExternalInput")
    wo_d = nc.dram_tensor("woT", [n_layers, GLU, H], BF16, kind="ExternalInput")
    wl_d = nc.dram_tensor("wl", [n_layers, H, 256], BF16, kind="ExternalInput")
    wm_d = nc.dram_tensor("wm", [n_layers, H, 256], BF16, kind="ExternalInput")
    hw_d = nc.dram_tensor("headT", [H, VOC], BF16, kind="ExternalInput")
    id_d = nc.dram_tensor("ident", [128, 128], BF16, kind="ExternalInput")
    ones_d = nc.dram_tensor("ones_k", [128, 1], F32R, kind="ExternalInput")
    onesr_d = nc.dram_tensor("ones_m", [1, 128], F32R, kind="ExternalInput")
    out_d = nc.dram_tensor("out", [VOC, S], F32, kind="ExternalOutput")

    from contextlib import ExitStack

    with tile.TileContext(nc) as tc, ExitStack() as ctx:
        singles = ctx.enter_context(tc.tile_pool(name="singles", bufs=1))
        wpool = ctx.enter_context(tc.tile_pool(name="wpool", bufs=2))
        hpool = ctx.enter_context(tc.tile_pool(name="hpool", bufs=1))
        npool = ctx.enter_context(tc.tile_pool(name="npool", bufs=2))
        apool = ctx.enter_context(tc.tile_pool(name="apool", bufs=2))
        ppool = ctx.enter_context(tc.tile_pool(name="ppool", bufs=2))
        gpool = ctx.enter_context(tc.tile_pool(name="gpool", bufs=2))
        ps_n = ctx.enter_context(tc.tile_pool(name="ps_n", bufs=1, space="PSUM"))
        ps_m = ctx.enter_context(tc.tile_pool(name="ps_m", bufs=2, space="PSUM"))
        ps_g = ctx.enter_context(tc.tile_pool(name="ps_g", bufs=2, space="PSUM"))
        ps_o = ctx.enter_context(tc.tile_pool(name="ps_o", bufs=2, space="PSUM"))

        # ---- constants / stem operands ----
        ident = singles.tile([128, 128], BF16, tag="ident")
        nc.sync.dma_start(out=ident, in_=id_d[:])
        ones_k_st = singles.tile([128, 1], F32R, tag="ones_k_st")
        nc.sync.dma_start(out=ones_k_st, in_=ones_d[:])
        ones_k = singles.tile([128, 1], F32R, tag="ones_k")
        ones_m_st = singles.tile([1, 128], F32R, tag="ones_m_st")
        nc.sync.dma_start(out=ones_m_st, in_=onesr_d[:])
        ones_m = singles.tile([1, 128], F32R, tag="ones_m")
        eps_sb = singles.tile([1, 1], F32, tag="eps")
        nc.vector.memset(eps_sb, float(EPS))

        x_st = singles.tile([128, 3, S], F32R, tag="x_st")
        nc.sync.dma_start(out=x_st, in_=x_d[:].rearrange("(t p) s -> p t s", p=128))
        x_sb = singles.tile([128, 3, S], F32R, tag="x")
        stw_st = singles.tile([128, 3, H], F32R, tag="stw_st")
        nc.sync.dma_start(out=stw_st, in_=stw_d[:].rearrange("(t p) s -> p t s", p=128))
        stw_sb = singles.tile([128, 3, H], F32R, tag="stw")

        # Route fp32r matmul operands through a DVE copy so each matmul's
        # operand has an engine writer (a matmul can carry only one
        # cross-engine wait through walrus codegen). Touch bf16 weight DMAs
        # with ldweights for the same reason.
        with nc.allow_low_precision(reason="fp32r staging copies"):
            nc.vector.tensor_copy(out=ones_k, in_=ones_k_st)
            nc.vector.tensor_copy(out=ones_m, in_=ones_m_st)
            nc.vector.tensor_copy(out=x_sb, in_=x_st)
            nc.vector.tensor_copy(out=stw_sb, in_=stw_st)
        nc.tensor.ldweights(ident[:, 0:128])

        # ---- h (resident, fp32r, one tile of 4 channel groups) ----
        h_all = hpool.tile([128, NT, S], F32R, tag="h", name="h")

        # ---- stem: h = stem_w @ x ----
        for tp in (0, 2):
            pst = ps_o.tile([128, 2, S], F32, tag="po")
            for t2 in (0, 1):
                for kt in range(3):
                    nc.tensor.matmul(
                        pst[:, t2, :],
                        stw_sb[:, kt, (tp + t2) * 128 : (tp + t2 + 1) * 128],
                        x_sb[:, kt, :],
                        start=(kt == 0),
                        stop=(kt == 2),
                    )
            with nc.allow_low_precision(reason="h fp32r copyback"):
                nc.scalar.copy(out=h_all[:, tp : tp + 2, :], in_=pst)

        def rms_rb():
            """Returns PSUM [128, S] fp32 broadcast of 1/sqrt(mean(h^2)+eps)."""
            sq = apool.tile([128, NT, S], F32R, tag="sq")
            nc.scalar.square(sq, h_all)
            ms = ps_n.tile([1, S], F32, tag="ms")
            for t in range(NT):
                nc.tensor.matmul(
                    ms,
                    ones_k[:, 0:1],
                    sq[:, t, :],
                    start=(t == 0),
                    stop=(t == NT - 1),
                )
            stdv = npool.tile([1, S], F32, tag="stdv")
            nc.scalar.activation(
                stdv, ms, AF.Sqrt, bias=eps_sb[0:1, 0:1], scale=1.0 / H
            )
            rstd = npool.tile([1, S], F32, tag="rstd")
            nc.vector.reciprocal_approx_fast(out=rstd, in_=stdv)
            rstd_r = npool.tile([1, S], F32R, tag="rstd_r")
            with nc.allow_low_precision(reason="fp32r rstd for broadcast matmul"):
                nc.vector.tensor_copy(out=rstd_r, in_=rstd)
            rb = ps_n.tile([128, S], F32, tag="rb")
            nc.tensor.matmul(
                rb,
                ones_m[0:1, :],
                rstd_r[:],
                start=True,
                stop=True,
            )
            return rb

        def pair_acc_and_residual(prod, tp):
            """Sum the pair's prod tile [128, (a 16, b 16, t2 2, k 16)] over k
            via 16 identity matmuls with a merged 2-dim rhs AP (N=512), giving
            acc[c, 32a+2b+t2]; then h[pair] += acc on DVE (strided read)."""
            acc = ps_m.tile([128, 2 * S], F32, tag="macc")
            for k in range(16):
                rhs = _bview(prod[:], k, [[512, 16], [16, 32]])
                nc.tensor.matmul(
                    acc[:],
                    ident[:],
                    rhs,
                    start=(k == 0),
                    stop=(k == 15),
                )
            hp = _bview(h_all[:], S * tp, [[256, 2], [16, 16], [1, 16]])
            acc_in = _bview(acc[:], 0, [[1, 2], [32, 16], [2, 16]])
            nc.vector.tensor_tensor(out=hp, in0=hp, in1=acc_in, op=ADD)

        for l in range(n_layers):
            wv_sb = wpool.tile([128, NT, GLU], BF16, tag="wv")
            nc.sync.dma_start(
                out=wv_sb, in_=wv_d[l].rearrange("(t p) o -> p t o", p=128)
            )
            wg_sb = wpool.tile([128, NT, GLU], BF16, tag="wg")
            nc.sync.dma_start(
                out=wg_sb, in_=wg_d[l].rearrange("(t p) o -> p t o", p=128)
            )
            wo_sb = wpool.tile([128, GT, H], BF16, tag="wo")
            nc.sync.dma_start(
                out=wo_sb, in_=wo_d[l].rearrange("(t p) c -> p t c", p=128)
            )
            wl_sb = wpool.tile([128, NT, 256], BF16, tag="wl")
            nc.sync.dma_start(
                out=wl_sb, in_=wl_d[l].rearrange("(t p) q -> p t q", p=128)
            )
            wm_sb = wpool.tile([128, NT, 256], BF16, tag="wm")
            nc.sync.dma_start(
                out=wm_sb, in_=wm_d[l].rearrange("(t p) q -> p t q", p=128)
            )
            nc.tensor.ldweights(wv_sb[:, 0, 0:128])
            nc.tensor.ldweights(wg_sb[:, 0, 0:128])
            nc.tensor.ldweights(wo_sb[:, 0, 0:128])

            # ---------- local mixer: out[c,16i+p] = sum_j Wl[c,p,j] u[c,16i+j]
            # prod free order (i, p, j): innermost j stride-1 for u, wl, out.
            rb = rms_rb()
            u_all = apool.tile([128, NT, S], BF16, tag="uall")
            nc.vector.tensor_tensor(
                out=u_all,
                in0=h_all,
                in1=_bview(rb[:], 0, [[0, NT], [1, S]]),
                op=MULT,
            )
            # prod memory layout [128, (i 16, p 16, t2 2, j 16)]
            for tp in (0, 2):
                prod = ppool.tile([128, 8192], BF16, tag="prod")
                for t2 in (0, 1):
                    u_t = u_all[:, tp + t2, :]
                    wl_t = wl_sb[:, tp + t2, :]
                    uv_d = _bview(u_t, 0, [[16, DV], [0, 16], [1, 16]])
                    wl_v = _bview(wl_t, 0, [[0, DV], [16, 16], [1, 16]])
                    out_d_ = _bview(
                        prod[:], 16 * t2, [[512, DV], [32, 16], [1, 16]]
                    )
                    nc.vector.tensor_tensor(
                        out=out_d_, in0=uv_d, in1=wl_v, op=MULT
                    )
                    uv_g = _bview(u_t, 16 * DV, [[16, 16 - DV], [0, 16], [1, 16]])
                    wl_g = _bview(wl_t, 0, [[0, 16 - DV], [16, 16], [1, 16]])
                    out_g = _bview(
                        prod[:], 512 * DV + 16 * t2, [[512, 16 - DV], [32, 16], [1, 16]]
                    )
                    nc.gpsimd.tensor_tensor(
                        out=out_g, in0=uv_g, in1=wl_g, op=MULT
                    )
                pair_acc_and_residual(prod, tp)

            # ---------- global mixer: out[c,16p+j] = sum_i Wg[c,p,i] v[c,16i+j]
            # vT[c,16j+i] written via transposed output AP; prod free (p, j, i).
            rb = rms_rb()
            vT_all = apool.tile([128, NT, S], BF16, tag="uall")
            nc.vector.tensor_tensor(
                out=_bview(vT_all[:], 0, [[256, NT], [16, 16], [1, 16]]),
                in0=_bview(h_all[:], 0, [[256, NT], [1, 16], [16, 16]]),
                in1=_bview(rb[:], 0, [[0, NT], [1, 16], [16, 16]]),
                op=MULT,
            )
            # prod memory layout [128, (p 16, j 16, t2 2, i 16)]
            for tp in (0, 2):
                prod = ppool.tile([128, 8192], BF16, tag="prod")
                for t2 in (0, 1):
                    vt_t = vT_all[:, tp + t2, :]
                    wm_t = wm_sb[:, tp + t2, :]
                    vt_d = _bview(vt_t, 0, [[0, DV], [16, 16], [1, 16]])
                    wm_v = _bview(wm_t, 0, [[16, DV], [0, 16], [1, 16]])
                    out_d_ = _bview(
                        prod[:], 16 * t2, [[512, DV], [32, 16], [1, 16]]
                    )
                    nc.vector.tensor_tensor(
                        out=out_d_, in0=vt_d, in1=wm_v, op=MULT
                    )
                    vt_g = _bview(vt_t, 0, [[0, 16 - DV], [16, 16], [1, 16]])
                    wm_g = _bview(
                        wm_t, 16 * DV, [[16, 16 - DV], [0, 16], [1, 16]]
                    )
                    out_g = _bview(
                        prod[:], 512 * DV + 16 * t2, [[512, 16 - DV], [32, 16], [1, 16]]
                    )
                    nc.gpsimd.tensor_tensor(
                        out=out_g, in0=vt_g, in1=wm_g, op=MULT
                    )
                pair_acc_and_residual(prod, tp)

            # ---------- GLU MLP
            rb = rms_rb()
            wn_all = apool.tile([128, NT, S], BF16, tag="wnall")
            nc.vector.tensor_tensor(
                out=wn_all,
                in0=h_all,
                in1=_bview(rb[:], 0, [[0, NT], [1, S]]),
                op=MULT,
            )
            gts = []
            for op_ in range(GT // 2):
                p1 = ps_g.tile([128, 2, S], F32, tag="pg")
                for oo in (0, 1):
                    for kt in range(NT):
                        nc.tensor.matmul(
                            p1[:, oo, :],
                            wv_sb[:, kt, (2 * op_ + oo) * 128 : (2 * op_ + oo + 1) * 128],
                            wn_all[:, kt, :],
                            start=(kt == 0),
                            stop=(kt == NT - 1),
                        )
                s1 = apool.tile([128, 2, S], BF16, tag="s1")
                if sim_compat:
                    sg = apool.tile([128, 2, S], BF16, tag="sg")
                    nc.scalar.activation(sg, p1, AF.Sigmoid)
                    nc.vector.tensor_tensor(out=s1, in0=sg, in1=p1, op=MULT)
                else:
                    nc.scalar.activation(s1, p1, AF.Silu)
                p3 = ps_g.tile([128, 2, S], F32, tag="pg")
                for oo in (0, 1):
                    for kt in range(NT):
                        nc.tensor.matmul(
                            p3[:, oo, :],
                            wg_sb[:, kt, (2 * op_ + oo) * 128 : (2 * op_ + oo + 1) * 128],
                            wn_all[:, kt, :],
                            start=(kt == 0),
                            stop=(kt == NT - 1),
                        )
                gt_ = gpool.tile([128, 2, S], BF16, tag=f"g{op_}")
                nc.vector.tensor_tensor(out=gt_, in0=s1, in1=p3, op=MULT)
                gts.append(gt_)
            for tp in (0, 2):
                po = ps_o.tile([128, 2, S], F32, tag="po")
                for t2 in (0, 1):
                    for ot in range(GT):
                        nc.tensor.matmul(
                            po[:, t2, :],
                            wo_sb[:, ot, (tp + t2) * 128 : (tp + t2 + 1) * 128],
                            gts[ot // 2][:, ot % 2, :],
                            start=(ot == 0),
                            stop=(ot == GT - 1),
                        )
                hp = h_all[:, tp : tp + 2, :]
                nc.vector.tensor_tensor(out=hp, in0=hp, in1=po[:], op=ADD)

        # ---------- head ----------
        hw_sb = singles.tile([128, NT, VOC], BF16, tag="hw")
        nc.sync.dma_start(out=hw_sb, in_=hw_d.rearrange("(t p) v -> p t v", p=128))
        nc.tensor.ldweights(hw_sb[:, 0, 0:128])
        rb = rms_rb()
        nrm_all = apool.tile([128, NT, S], BF16, tag="wnall")
        nc.vector.tensor_tensor(
            out=nrm_all,
            in0=h_all,
            in1=_bview(rb[:], 0, [[0, NT], [1, S]]),
            op=MULT,
        )
        po = ps_o.tile([128, 2, S], F32, tag="po")
        for mc in range(VOC // 128):
            for kt in range(NT):
                nc.tensor.matmul(
                    po[:, mc, :],
                    hw_sb[:, kt, mc * 128 : (mc + 1) * 128],
                    nrm_all[:, kt, :],
                    start=(kt == 0),
                    stop=(kt == NT - 1),
                )
        ot_sb = apool.tile([128, 2, S], F32, tag="osb")
        nc.scalar.copy(out=ot_sb, in_=po)
        for mc in range(VOC // 128):
            nc.sync.dma_start(
                out=out_d[mc * 128 : (mc + 1) * 128, :], in_=ot_sb[:, mc, :]
            )

    nc.compile()
    return nc


def _prep_inputs(inputs, n_layers=L):
    """Host-side weight folding + layout prep. Returns dict of np arrays."""
    f = lambda k: np.asarray(inputs[k], dtype=np.float32)
    x = f("x")
    stem_w = f("stem_w")  # [H, CIN]
    rl, rg, rf = f("rms_local"), f("rms_global"), f("rms_ffn")
    al, ag, am = f("alpha_local"), f("alpha_global"), f("alpha_mlp")
    w_local, w_global = f("w_local"), f("w_global")  # [L, H, BLK, BLK]
    wv, wg, wo = f("wv"), f("wg"), f("wo")
    head_rms, head_w = f("head_rms"), f("head_w")
    hls = np.float32(np.asarray(inputs["head_logit_scale"]))

    bf = ml_dtypes.bfloat16
    nl = n_layers

    # local: fold alpha_local * rms_local[c] into Wl[c,p,j]; layout [c, 16p+j]
    wl_h = (w_local[:nl] * al[:nl, None, None, None] * rl[:nl, :, None, None]).reshape(
        nl, H, 256
    )
    # global: Wg[c,p,i]; layout [c, 16p+i]
    wm_h = (w_global[:nl] * ag[:nl, None, None, None] * rg[:nl, :, None, None]).reshape(
        nl, H, 256
    )
    # GLU: fold rms_ffn into wv/wg columns; alpha_mlp into wo
    wvT = np.ascontiguousarray(
        np.transpose(wv[:nl] * rf[:nl, None, :], (0, 2, 1))
    )  # [L, H, GLU]
    wgT = np.ascontiguousarray(np.transpose(wg[:nl] * rf[:nl, None, :], (0, 2, 1)))
    woT = np.ascontiguousarray(
        np.transpose(wo[:nl] * am[:nl, None, None], (0, 2, 1))
    )  # [L, GLU, H]
    headT = np.ascontiguousarray((head_w * head_rms[None, :] * hls).T)  # [H, VOC]

    stw_pad = np.zeros((384, H), np.float32)
    stw_pad[:CIN] = stem_w.T
    common = {
        "stem_wT": stw_pad,  # [384, H] zero-padded
        "wvT": wvT.astype(bf),
        "wgT": wgT.astype(bf),
        "woT": woT.astype(bf),
        "wl": wl_h.astype(bf),
        "wm": wm_h.astype(bf),
        "headT": headT.astype(bf),
        "ident": np.eye(128, dtype=bf),
        "ones_k": np.ones((128, 1), np.float32),
        "ones_m": np.ones((1, 128), np.float32),
    }
    per_core = []
    for b in range(B):
        xp = np.zeros((384, S), np.float32)
        xp[:CIN] = x[b, :, 0, :]
        per_core.append(dict(common, x=xp))
    return per_core


def run(inputs, n_layers=L, trace=False):
    key = n_layers
    if key not in _PROG_CACHE:
        _PROG_CACHE[key] = build_program(n_layers)
    nc = _PROG_CACHE[key]
    in_maps = _prep_inputs(inputs, n_layers)
    res = run_bass_kernel_spmd(nc, in_maps, core_ids=list(range(B)), trace=trace)
    out = np.stack([r["out"] for r in res.results])  # [B, VOC, S]
    return out[:, :, None, :].astype(np.float32), res


def kernel(**inputs):
    out, _ = run(inputs, L, trace=False)
    return out


# revision 16
# speedup vs baseline: 1.1203x; 1.1190x over previous
"""Trainium2 Bass kernel for nn_ByteGridModel (dense_cnn).

Sharding: pure data-parallel over batch B=8 -> 8 cores, one batch item per
core, no collectives. Weights replicated (streamed per layer, double
buffered).

Per-core layout: channels on partitions, h = [128, NT=4, S=256] fp32r
resident in SBUF (one tile, four 128-channel groups).

v2: mixer broadcast-products reordered so every operand's innermost free
    dim is stride-1 bf16 -> DVE 2x_1P mode; ACT sqrt + DVE fast-reciprocal
    for rmsnorm; transposed-v trick for the global mixer.
v3: PE became the bottleneck (serial LDWEIGHTS per matmul + power-governor
    duty cycle capping PE at ~1.37GHz avg). Halve matmul count: process
    h-tile PAIRS with N=512 accumulation psums; residual adds on DVE
    (reading the [128,512] psum once); merge per-tile DVE muls and ACT
    squares into single wide ops.
"""

import numpy as np
import ml_dtypes

import concourse.bacc as bacc
import concourse.bass as bass
import concourse.tile as tile
import concourse.mybir as mybir
from concourse.bass_utils import run_bass_kernel_spmd

B, S, H, GLU, VOC, L, CIN, BLK = 8, 256, 512, 1024, 256, 24, 320, 16
EPS = 1e-5
NT = H // 128  # 4 channel tiles
GT = GLU // 128  # 8 glu tiles
DV = 16  # of the 16 outer slices of each product tile, DVE does DV, GPSIMD 16-DV

F32 = mybir.dt.float32
F32R = mybir.dt.float32r
BF16 = mybir.dt.bfloat16
MULT = mybir.AluOpType.mult
ADD = mybir.AluOpType.add
AF = mybir.ActivationFunctionType

_PROG_CACHE = {}


def _bview(base, doff, free_dims):
    """View of a 2D sbuf AP with custom (possibly broadcast) free dims."""
    return bass.AP(
        tensor=base.tensor,
        offset=base.offset + doff,
        ap=[list(base.ap[0])] + [list(d) for d in free_dims],
    )


def build_program(n_layers=L, sim_compat=False):
    nc = bacc.Bacc("TRN2")

    x_d = nc.dram_tensor("x", [384, S], F32R, kind="ExternalInput")
    stw_d = nc.dram_tensor("stem_wT", [384, H], F32R, kind="ExternalInput")
    wv_d = nc.dram_tensor("wvT", [n_layers, H, GLU], BF16, kind="ExternalInput")
    wg_d = nc.dram_tensor("wgT", [n_layers, H, GLU], BF16, kind="ExternalInput")
    wo_d = nc.dram_tensor("woT", [n_layers, GLU, H], BF16, kind="ExternalInput")
    wl_d = nc.dram_tensor("wl", [n_layers, H, 256], BF16, kind="ExternalInput")
    wm_d = nc.dram_tensor("wm", [n_layers, H, 256], BF16, kind="ExternalInput")
    hw_d = nc.dram_tensor("headT", [H, VOC], BF16, kind="ExternalInput")
    ones_d = nc.dram_tensor("ones_k", [128, 1], F32R, kind="ExternalInput")
    onesr_d = nc.dram_tensor("ones_m", [1, 128], F32R, kind="ExternalInput")
    out_d = nc.dram_tensor("out", [VOC, S], F32, kind="ExternalOutput")

    from contextlib import ExitStack

    with tile.TileContext(nc) as tc, ExitStack() as ctx:
        singles = ctx.enter_context(tc.tile_pool(name="singles", bufs=1))
        wpool = ctx.enter_context(tc.tile_pool(name="wpool", bufs=2))
        hpool = ctx.enter_context(tc.tile_pool(name="hpool", bufs=1))
        npool = ctx.enter_context(tc.tile_pool(name="npool", bufs=2))
        apool = ctx.enter_context(tc.tile_pool(name="apool", bufs=2))
        ppool = ctx.enter_context(tc.tile_pool(name="ppool", bufs=2))
        gpool = ctx.enter_context(tc.tile_pool(name="gpool", bufs=2))
        ps_n = ctx.enter_context(tc.tile_pool(name="ps_n", bufs=1, space="PSUM"))
        ps_g = ctx.enter_context(tc.tile_pool(name="ps_g", bufs=2, space="PSUM"))
        ps_o = ctx.enter_context(tc.tile_pool(name="ps_o", bufs=2, space="PSUM"))

        # ---- constants / stem operands ----
        ones_k_st = singles.tile([128, 1], F32R, tag="ones_k_st")
        nc.sync.dma_start(out=ones_k_st, in_=ones_d[:])
        ones_k = singles.tile([128, 1], F32R, tag="ones_k")
        ones_m_st = singles.tile([1, 128], F32R, tag="ones_m_st")
        nc.sync.dma_start(out=ones_m_st, in_=onesr_d[:])
        ones_m = singles.tile([1, 128], F32R, tag="ones_m")
        eps_sb = singles.tile([1, 1], F32, tag="eps")
        nc.vector.memset(eps_sb, float(EPS))

        x_st = singles.tile([128, 3, S], F32R, tag="x_st")
        nc.sync.dma_start(out=x_st, in_=x_d[:].rearrange("(t p) s -> p t s", p=128))
        x_sb = singles.tile([128, 3, S], F32R, tag="x")
        stw_st = singles.tile([128, 3, H], F32R, tag="stw_st")
        nc.sync.dma_start(out=stw_st, in_=stw_d[:].rearrange("(t p) s -> p t s", p=128))
        stw_sb = singles.tile([128, 3, H], F32R, tag="stw")

        # Route fp32r matmul operands through a DVE copy so each matmul's
        # operand has an engine writer (a matmul can carry only one
        # cross-engine wait through walrus codegen). Touch bf16 weight DMAs
        # with ldweights for the same reason.
        with nc.allow_low_precision(reason="fp32r staging copies"):
            nc.vector.tensor_copy(out=ones_k, in_=ones_k_st)
            nc.vector.tensor_copy(out=ones_m, in_=ones_m_st)
            nc.vector.tensor_copy(out=x_sb, in_=x_st)
            nc.vector.tensor_copy(out=stw_sb, in_=stw_st)

        # ---- h (resident, fp32r, one tile of 4 channel groups) ----
        h_all = hpool.tile([128, NT, S], F32R, tag="h", name="h")

        # ---- stem: h = stem_w @ x ----
        for tp in (0, 2):
            pst = ps_o.tile([128, 2, S], F32, tag="po")
            for t2 in (0, 1):
                for kt in range(3):
                    nc.tensor.matmul(
                        pst[:, t2, :],
                        stw_sb[:, kt, (tp + t2) * 128 : (tp + t2 + 1) * 128],
                        x_sb[:, kt, :],
                        start=(kt == 0),
                        stop=(kt == 2),
                    )
            with nc.allow_low_precision(reason="h fp32r copyback"):
                nc.scalar.copy(out=h_all[:, tp : tp + 2, :], in_=pst)

        def rms_rb():
            """Returns PSUM [128, S] fp32 broadcast of 1/sqrt(mean(h^2)+eps)."""
            sq = apool.tile([128, NT, S], F32R, tag="sq")
            nc.scalar.square(sq, h_all)
            ms = ps_n.tile([1, S], F32, tag="ms")
            for t in range(NT):
                nc.tensor.matmul(
                    ms,
                    ones_k[:, 0:1],
                    sq[:, t, :],
                    start=(t == 0),
                    stop=(t == NT - 1),
                )
            stdv = npool.tile([1, S], F32, tag="stdv")
            nc.scalar.activation(
                stdv, ms, AF.Sqrt, bias=eps_sb[0:1, 0:1], scale=1.0 / H
            )
            rstd = npool.tile([1, S], F32, tag="rstd")
            nc.vector.reciprocal_approx_fast(out=rstd, in_=stdv)
            rstd_r = npool.tile([1, S], F32R, tag="rstd_r")
            with nc.allow_low_precision(reason="fp32r rstd for broadcast matmul"):
                nc.vector.tensor_copy(out=rstd_r, in_=rstd)
            rb = ps_n.tile([128, S], F32, tag="rb")
            nc.tensor.matmul(
                rb,
                ones_m[0:1, :],
                rstd_r[:],
                start=True,
                stop=True,
            )
            return rb

        def pair_acc_and_residual(prod, tp):
            """Sum the pair's prod tile [128, (a 16, b 16, t2 2, k 16)] over k
            with an add-tree on unthrottled engines (PE is power-duty-capped):
            GPSIMD does the big first level, DVE the rest, all bf16 2x-mode;
            then h[pair] += result on DVE (strided read)."""
            t1 = ppool.tile([128, 4096], BF16, tag="t1")
            nc.gpsimd.tensor_tensor(
                out=_bview(t1[:], 0, [[16, 256], [8, 2], [1, 8]]),
                in0=_bview(prod[:], 0, [[32, 256], [16, 2], [1, 8]]),
                in1=_bview(prod[:], 8, [[32, 256], [16, 2], [1, 8]]),
                op=ADD,
            )
            t2_ = ppool.tile([128, 2048], BF16, tag="t2")
            nc.vector.tensor_tensor(
                out=_bview(t2_[:], 0, [[8, 256], [4, 2], [1, 4]]),
                in0=_bview(t1[:], 0, [[16, 256], [8, 2], [1, 4]]),
                in1=_bview(t1[:], 4, [[16, 256], [8, 2], [1, 4]]),
                op=ADD,
            )
            t3 = ppool.tile([128, 1024], BF16, tag="t3")
            nc.vector.tensor_tensor(
                out=_bview(t3[:], 0, [[4, 256], [2, 2], [1, 2]]),
                in0=_bview(t2_[:], 0, [[8, 256], [4, 2], [1, 2]]),
                in1=_bview(t2_[:], 2, [[8, 256], [4, 2], [1, 2]]),
                op=ADD,
            )
            t4 = ppool.tile([128, 512], BF16, tag="t4")
            nc.vector.tensor_tensor(
                out=_bview(t4[:], 0, [[2, 256], [1, 2]]),
                in0=_bview(t3[:], 0, [[4, 256], [2, 2]]),
                in1=_bview(t3[:], 1, [[4, 256], [2, 2]]),
                op=ADD,
            )
            hp = _bview(h_all[:], S * tp, [[256, 2], [16, 16], [1, 16]])
            t4_in = _bview(t4[:], 0, [[1, 2], [32, 16], [2, 16]])
            nc.vector.tensor_tensor(out=hp, in0=hp, in1=t4_in, op=ADD)

        for l in range(n_layers):
            wv_sb = wpool.tile([128, NT, GLU], BF16, tag="wv")
            nc.sync.dma_start(
                out=wv_sb, in_=wv_d[l].rearrange("(t p) o -> p t o", p=128)
            )
            wg_sb = wpool.tile([128, NT, GLU], BF16, tag="wg")
            nc.sync.dma_start(
                out=wg_sb, in_=wg_d[l].rearrange("(t p) o -> p t o", p=128)
            )
            wo_sb = wpool.tile([128, GT, H], BF16, tag="wo")
            nc.sync.dma_start(
                out=wo_sb, in_=wo_d[l].rearrange("(t p) c -> p t c", p=128)
            )
            wl_sb = wpool.tile([128, NT, 256], BF16, tag="wl")
            nc.sync.dma_start(
                out=wl_sb, in_=wl_d[l].rearrange("(t p) q -> p t q", p=128)
            )
            wm_sb = wpool.tile([128, NT, 256], BF16, tag="wm")
            nc.sync.dma_start(
                out=wm_sb, in_=wm_d[l].rearrange("(t p) q -> p t q", p=128)
            )
            nc.tensor.ldweights(wv_sb[:, 0, 0:128])
            nc.tensor.ldweights(wg_sb[:, 0, 0:128])
            nc.tensor.ldweights(wo_sb[:, 0, 0:128])

            # ---------- local mixer: out[c,16i+p] = sum_j Wl[c,p,j] u[c,16i+j]
            # prod free order (i, p, j): innermost j stride-1 for u, wl, out.
            rb = rms_rb()
            u_all = apool.tile([128, NT, S], BF16, tag="uall")
            nc.vector.tensor_tensor(
                out=u_all,
                in0=h_all,
                in1=_bview(rb[:], 0, [[0, NT], [1, S]]),
                op=MULT,
            )
            # prod memory layout [128, (i 16, p 16, t2 2, j 16)]
            for tp in (0, 2):
                prod = ppool.tile([128, 8192], BF16, tag="prod")
                for t2 in (0, 1):
                    u_t = u_all[:, tp + t2, :]
                    wl_t = wl_sb[:, tp + t2, :]
                    uv_d = _bview(u_t, 0, [[16, DV], [0, 16], [1, 16]])
                    wl_v = _bview(wl_t, 0, [[0, DV], [16, 16], [1, 16]])
                    out_d_ = _bview(
                        prod[:], 16 * t2, [[512, DV], [32, 16], [1, 16]]
                    )
                    nc.vector.tensor_tensor(
                        out=out_d_, in0=uv_d, in1=wl_v, op=MULT
                    )
                    if DV < 16:
                        uv_g = _bview(u_t, 16 * DV, [[16, 16 - DV], [0, 16], [1, 16]])
                        wl_g = _bview(wl_t, 0, [[0, 16 - DV], [16, 16], [1, 16]])
                        out_g = _bview(
                            prod[:], 512 * DV + 16 * t2,
                            [[512, 16 - DV], [32, 16], [1, 16]],
                        )
                        nc.gpsimd.tensor_tensor(
                            out=out_g, in0=uv_g, in1=wl_g, op=MULT
                        )
                pair_acc_and_residual(prod, tp)

            # ---------- global mixer: out[c,16p+j] = sum_i Wg[c,p,i] v[c,16i+j]
            # vT[c,16j+i] written via transposed output AP; prod free (p, j, i).
            rb = rms_rb()
            vT_all = apool.tile([128, NT, S], BF16, tag="uall")
            nc.vector.tensor_tensor(
                out=_bview(vT_all[:], 0, [[256, NT], [16, 16], [1, 16]]),
                in0=_bview(h_all[:], 0, [[256, NT], [1, 16], [16, 16]]),
                in1=_bview(rb[:], 0, [[0, NT], [1, 16], [16, 16]]),
                op=MULT,
            )
            # prod memory layout [128, (p 16, j 16, t2 2, i 16)]
            for tp in (0, 2):
                prod = ppool.tile([128, 8192], BF16, tag="prod")
                for t2 in (0, 1):
                    vt_t = vT_all[:, tp + t2, :]
                    wm_t = wm_sb[:, tp + t2, :]
                    vt_d = _bview(vt_t, 0, [[0, DV], [16, 16], [1, 16]])
                    wm_v = _bview(wm_t, 0, [[16, DV], [0, 16], [1, 16]])
                    out_d_ = _bview(
                        prod[:], 16 * t2, [[512, DV], [32, 16], [1, 16]]
                    )
                    nc.vector.tensor_tensor(
                        out=out_d_, in0=vt_d, in1=wm_v, op=MULT
                    )
                    if DV < 16:
                        vt_g = _bview(vt_t, 0, [[0, 16 - DV], [16, 16], [1, 16]])
                        wm_g = _bview(
                            wm_t, 16 * DV, [[16, 16 - DV], [0, 16], [1, 16]]
                        )
                        out_g = _bview(
                            prod[:], 512 * DV + 16 * t2,
                            [[512, 16 - DV], [32, 16], [1, 16]],
                        )
                        nc.gpsimd.tensor_tensor(
                            out=out_g, in0=vt_g, in1=wm_g, op=MULT
                        )
                pair_acc_and_residual(prod, tp)

            # ---------- GLU MLP
            rb = rms_rb()
            wn_all = apool.tile([128, NT, S], BF16, tag="wnall")
            nc.vector.tensor_tensor(
                out=wn_all,
                in0=h_all,
                in1=_bview(rb[:], 0, [[0, NT], [1, S]]),
                op=MULT,
            )
            gts = []
            for op_ in range(GT // 2):
                p1 = ps_g.tile([128, 2, S], F32, tag="pg")
                for oo in (0, 1):
                    for kt in range(NT):
                        nc.tensor.matmul(
                            p1[:, oo, :],
                            wv_sb[:, kt, (2 * op_ + oo) * 128 : (2 * op_ + oo + 1) * 128],
                            wn_all[:, kt, :],
                            start=(kt == 0),
                            stop=(kt == NT - 1),
                        )
                s1 = apool.tile([128, 2, S], BF16, tag="s1")
                if sim_compat:
                    sg = apool.tile([128, 2, S], BF16, tag="sg")
                    nc.scalar.activation(sg, p1, AF.Sigmoid)
                    nc.vector.tensor_tensor(out=s1, in0=sg, in1=p1, op=MULT)
                else:
                    nc.scalar.activation(s1, p1, AF.Silu)
                p3 = ps_g.tile([128, 2, S], F32, tag="pg")
                for oo in (0, 1):
                    for kt in range(NT):
                        nc.tensor.matmul(
                            p3[:, oo, :],
                            wg_sb[:, kt, (2 * op_ + oo) * 128 : (2 * op_ + oo + 1) * 128],
                            wn_all[:, kt, :],
                            start=(kt == 0),
                            stop=(kt == NT - 1),
                        )
                gt_ = gpool.tile([128, 2, S], BF16, tag=f"g{op_}")
                nc.vector.tensor_tensor(out=gt_, in0=s1, in1=p3, op=MULT)
                gts.append(gt_)
            for tp in (0, 2):
                po = ps_o.tile([128, 2, S], F32, tag="po")
                for t2 in (0, 1):
                    for ot in range(GT):
                        nc.tensor.matmul(
                            po[:, t2, :],
                            wo_sb[:, ot, (tp + t2) * 128 : (tp + t2 + 1) * 128],
                            gts[ot // 2][:, ot % 2, :],
                            start=(ot == 0),
                            stop=(ot == GT - 1),
                        )
                hp = h_all[:, tp : tp + 2, :]
                nc.vector.tensor_tensor(out=hp, in0=hp, in1=po[:], op=ADD)

        # ---------- head ----------
        hw_sb = singles.tile([128, NT, VOC], BF16, tag="hw")
        nc.sync.dma_start(out=hw_sb, in_=hw_d.rearrange("(t p) v -> p t v", p=128))
        nc.tensor.ldweights(hw_sb[:, 0, 0:128])
        rb = rms_rb()
        nrm_all = apool.tile([128, NT, S], BF16, tag="wnall")
        nc.vector.tensor_tensor(
            out=nrm_all,
            in0=h_all,
            in1=_bview(rb[:], 0, [[0, NT], [1, S]]),
            op=MULT,
        )
        po = ps_o.tile([128, 2, S], F32, tag="po")
        for mc in range(VOC // 128):
            for kt in range(NT):
                nc.tensor.matmul(
                    po[:, mc, :],
                    hw_sb[:, kt, mc * 128 : (mc + 1) * 128],
                    nrm_all[:, kt, :],
                    start=(kt == 0),
                    stop=(kt == NT - 1),
                )
        ot_sb = apool.tile([128, 2, S], F32, tag="osb")
        nc.scalar.copy(out=ot_sb, in_=po)
        for mc in range(VOC // 128):
            nc.sync.dma_start(
                out=out_d[mc * 128 : (mc + 1) * 128, :], in_=ot_sb[:, mc, :]
            )

    nc.compile()
    return nc


def _prep_inputs(inputs, n_layers=L):
    """Host-side weight folding + layout prep. Returns dict of np arrays."""
    f = lambda k: np.asarray(inputs[k], dtype=np.float32)
    x = f("x")
    stem_w = f("stem_w")  # [H, CIN]
    rl, rg, rf = f("rms_local"), f("rms_global"), f("rms_ffn")
    al, ag, am = f("alpha_local"), f("alpha_global"), f("alpha_mlp")
    w_local, w_global = f("w_local"), f("w_global")  # [L, H, BLK, BLK]
    wv, wg, wo = f("wv"), f("wg"), f("wo")
    head_rms, head_w = f("head_rms"), f("head_w")
    hls = np.float32(np.asarray(inputs["head_logit_scale"]))

    bf = ml_dtypes.bfloat16
    nl = n_layers

    # local: fold alpha_local * rms_local[c] into Wl[c,p,j]; layout [c, 16p+j]
    wl_h = (w_local[:nl] * al[:nl, None, None, None] * rl[:nl, :, None, None]).reshape(
        nl, H, 256
    )
    # global: Wg[c,p,i]; layout [c, 16p+i]
    wm_h = (w_global[:nl] * ag[:nl, None, None, None] * rg[:nl, :, None, None]).reshape(
        nl, H, 256
    )
    # GLU: fold rms_ffn into wv/wg columns; alpha_mlp into wo
    wvT = np.ascontiguousarray(
        np.transpose(wv[:nl] * rf[:nl, None, :], (0, 2, 1))
    )  # [L, H, GLU]
    wgT = np.ascontiguousarray(np.transpose(wg[:nl] * rf[:nl, None, :], (0, 2, 1)))
    woT = np.ascontiguousarray(
        np.transpose(wo[:nl] * am[:nl, None, None], (0, 2, 1))
    )  # [L, GLU, H]
    headT = np.ascontiguousarray((head_w * head_rms[None, :] * hls).T)  # [H, VOC]

    stw_pad = np.zeros((384, H), np.float32)
    stw_pad[:CIN] = stem_w.T
    common = {
        "stem_wT": stw_pad,  # [384, H] zero-padded
        "wvT": wvT.astype(bf),
        "wgT": wgT.astype(bf),
        "woT": woT.astype(bf),
        "wl": wl_h.astype(bf),
        "wm": wm_h.astype(bf),
        "headT": headT.astype(bf),
        "ones_k": np.ones((128, 1), np.float32),
        "ones_m": np.ones((1, 128), np.float32),
    }
    per_core = []
    for b in range(B):
        xp = np.zeros((384, S), np.float32)
        xp[:CIN] = x[b, :, 0, :]
        per_core.append(dict(common, x=xp))
    return per_core


def run(inputs, n_layers=L, trace=False):
    key = n_layers
    if key not in _PROG_CACHE:
        _PROG_CACHE[key] = build_program(n_layers)
    nc = _PROG_CACHE[key]
    in_maps = _prep_inputs(inputs, n_layers)
    res = run_bass_kernel_spmd(nc, in_maps, core_ids=list(range(B)), trace=trace)
    out = np.stack([r["out"] for r in res.results])  # [B, VOC, S]
    return out[:, :, None, :].astype(np.float32), res


def kernel(**inputs):
    out, _ = run(inputs, L, trace=False)
    return out
